# revision 26
# baseline (speedup 1.0000x reference)
"""DialogueRNNCell fused Bass kernel for 8 Trainium2 NeuronCores.

Sharding: hidden-dim (column) sharding for all GRU/dense weights, batch
sharding for the attention over g_hist, dense1-3 replicated over batch
shards. Two AllGathers knit the stages together (c_t/c_a/c_v after
dense1-3, q_sel after the p-GRU).

All activations live transposed on-chip: (feature partitions, batch free).
Matmuls run in bf16 with f32 PSUM accumulation.
"""
import numpy as np
import ml_dtypes

N = 8
B, P, T = 256, 2, 128
D = 1024
BL = B // N      # 32 batch rows per core
H = D // N       # 128 hidden cols per core
G3 = 3 * H       # 384 gate rows per core
CH = 2           # ghist chunk size (batch rows per DMA)
DIN = {'t': 768, 'v': 512, 'a': 300}
KTU = {'t': 6, 'v': 4, 'a': 3}          # padded K-tiles for dense_U
MODS = ('t', 'v', 'a')
BF = ml_dtypes.bfloat16

# attention wiring: src -> att-w indices feeding (k=0,1,2)
SRC_ATT = {'t': [0, 3, 5], 'v': [1, 6, 8], 'a': [2, 4, 7]}
# receiver m -> 3 slots (src, k) in reference stack order [c_mm, c_mx, c_my]
RECV_SLOTS = {
    't': [('t', 0), ('a', 1), ('v', 1)],
    'a': [('a', 0), ('t', 1), ('v', 2)],
    'v': [('v', 0), ('t', 2), ('a', 2)],
}
RECV_SA = {'t': 'sa1', 'a': 'sa2', 'v': 'sa3'}
RECV_DENSE = {'t': 'dense1', 'a': 'dense2', 'v': 'dense3'}


def pos_enc_table():
    pos = np.arange(T, dtype=np.float32)[:, None]
    div = np.exp(np.arange(0, D, 2, dtype=np.float32) * (-np.log(10000.0) / D))
    ang = pos * div
    return np.stack([np.sin(ang), np.cos(ang)], axis=-1).reshape(T, D).astype(np.float32)


def _bf(x):
    return np.ascontiguousarray(np.asarray(x, np.float32).astype(BF))


def _f32(x):
    return np.ascontiguousarray(np.asarray(x, np.float32))


def _ktiles(mat_T, kt):
    """(Din, M) -> zero-padded (kt, 128, M) K-tile stack."""
    din, m = mat_T.shape
    out = np.zeros((kt * 128, m), np.float32)
    out[:din] = mat_T
    return out.reshape(kt, 128, m)


def _gate_major(wT_tiles):
    """(kt, 128, 384) -> (3, kt, 128, 128)."""
    kt = wT_tiles.shape[0]
    return wT_tiles.reshape(kt, 128, 3, 128).transpose(2, 0, 1, 3)


def prep_inputs(Ut, Uv, Ua, qmask, g_hist_t, g_hist_v, g_hist_a,
                q0_t, q0_v, q0_a, e0_t, e0_v, e0_a, params):
    """Build the 8 per-core input dicts (all numpy, host-side only)."""
    U = {'t': _f32(Ut), 'v': _f32(Uv), 'a': _f32(Ua)}
    qmask = _f32(qmask)
    ghist = {'t': _f32(g_hist_t), 'v': _f32(g_hist_v), 'a': _f32(g_hist_a)}
    q0 = {'t': _f32(q0_t), 'v': _f32(q0_v), 'a': _f32(q0_a)}
    e0 = {'t': _f32(e0_t), 'v': _f32(e0_v), 'a': _f32(e0_a)}
    pe = pos_enc_table()
    att = [_f32(w) for w in params['att']]

    shared = {}
    for m in MODS:
        Wd, bd = _f32(params[f'dense_{m}'][0]), _f32(params[f'dense_{m}'][1])
        shared[f'wd_{m}'] = _bf(_ktiles(Wd.T, KTU[m]))              # (kt,128,1024)
        shared[f'bd_{m}'] = _f32(bd.reshape(8, 128).T)              # (128, 8)
        shared[f'u_{m}'] = _bf(_ktiles(U[m].T, KTU[m]))             # (kt,128,256)
        W3 = np.stack([att[i] for i in SRC_ATT[m]], axis=1)         # (1024, 3)
        shared[f'w3_{m}'] = _bf(W3.reshape(8, 128, 3))
        w3pad = np.zeros((8, 128, 32), np.float32)
        w3pad[:, :, 0:3] = W3.reshape(8, 128, 3)
        shared[f'w3p_{m}'] = _bf(w3pad)                             # (8,128,32)
        shared[f'hprev_{m}'] = _bf(ghist[m][-1].T.reshape(8, 128, B))
        shared[f'q0b_{m}'] = _bf(q0[m].transpose(1, 2, 0).reshape(P, 8, 128, B))
        shared[f'e0b_{m}'] = _bf(e0[m].T.reshape(8, 128, B))
        wsa = _f32(params[RECV_SA[m]][0])
        shared[f'wsa_{m}'] = _bf(wsa.reshape(8, 128, 1))
        W1, b1 = _f32(params[RECV_DENSE[m]][0]), _f32(params[RECV_DENSE[m]][1])
        shared[f'w1_{m}'] = _bf(W1.T.reshape(24, 128, 1024))
        shared[f'b1_{m}'] = _f32(b1.reshape(8, 128).T)              # (128, 8)
    shared['pe_t'] = _bf(pe)                                        # (128, 1024)
    shared['pe_T'] = _bf(pe.T.reshape(8, 128, T))                   # (8,128,128)
    shared['qm0'] = _f32(qmask[:, 0].reshape(1, B))
    shared['qm1'] = _f32(qmask[:, 1].reshape(1, B))
    sab = np.concatenate([np.full(3 * BL, float(_f32(params[RECV_SA[m]][1]).reshape(-1)[0]), np.float32)
                          for m in MODS])
    shared['sab'] = sab.reshape(1, 3 * 3 * BL)

    in_maps = []
    for r in range(N):
        d = dict(shared)
        hs = np.arange(H * r, H * (r + 1))
        sel = np.concatenate([hs, D + hs, 2 * D + hs])
        for m in MODS:
            for gru, ktw in (('g', 16), ('p', 16), ('e', 8)):
                w_ih, w_hh, b_ih, b_hh = [_f32(x) for x in params[f'{gru}_{m}']]
                d[f'w{gru}i_{m}'] = _bf(_gate_major(_ktiles(w_ih[sel].T, ktw)))
                d[f'w{gru}h_{m}'] = _bf(_gate_major(_ktiles(w_hh[sel].T, 8)))
                bi, bh = b_ih[sel], b_hh[sel]
                bias4 = np.stack([(bi + bh)[:H], (bi + bh)[H:2 * H],
                                  bi[2 * H:], bh[2 * H:]], axis=1)   # (128, 4)
                d[f'b{gru}_{m}'] = _f32(bias4)
            d[f'hprevs_{m}'] = _f32(ghist[m][-1].T[hs])              # (128, 256)
            d[f'q0s_{m}'] = _f32(q0[m].transpose(1, 2, 0)[:, hs])    # (2, 128, 256)
            d[f'e0s_{m}'] = _f32(e0[m].T[hs])                        # (128, 256)
            gh_loc = ghist[m][:, BL * r:BL * (r + 1), :]             # (T, 32, D)
            d[f'ght_{m}'] = _bf(gh_loc.transpose(1, 0, 2))           # (32,128,1024)
        in_maps.append(d)
    return in_maps


_BUILT = None


def build_module():
    global _BUILT
    if _BUILT is not None:
        return _BUILT
    import concourse.bass as bass
    import concourse.mybir as mybir
    import concourse.tile as tile
    from concourse import bacc

    nc = bacc.Bacc("TRN2", num_devices=N, debug=False)
    f32, bf16 = mybir.dt.float32, mybir.dt.bfloat16

    di = {}
    def dram_in(name, shape, dt=bf16):
        di[name] = nc.dram_tensor(name, list(shape), dt, kind="ExternalInput")

    for m in MODS:
        dram_in(f'wd_{m}', (KTU[m], 128, D)); dram_in(f'bd_{m}', (128, 8), f32)
        dram_in(f'u_{m}', (KTU[m], 128, B))
        dram_in(f'w3_{m}', (8, 128, 3)); dram_in(f'w3p_{m}', (8, 128, 32))
        dram_in(f'hprev_{m}', (8, 128, B)); dram_in(f'q0b_{m}', (P, 8, 128, B))
        dram_in(f'e0b_{m}', (8, 128, B))
        dram_in(f'wsa_{m}', (8, 128, 1)); dram_in(f'w1_{m}', (24, 128, D))
        dram_in(f'b1_{m}', (128, 8), f32)
        for gru, ktw in (('g', 16), ('p', 16), ('e', 8)):
            dram_in(f'w{gru}i_{m}', (3, ktw, 128, 128))
            dram_in(f'w{gru}h_{m}', (3, 8, 128, 128))
            dram_in(f'b{gru}_{m}', (128, 4), f32)
        dram_in(f'hprevs_{m}', (128, B), f32)
        dram_in(f'q0s_{m}', (P, 128, B), f32)
        dram_in(f'e0s_{m}', (128, B), f32)
        dram_in(f'ght_{m}', (BL, 128, D))
    dram_in('pe_t', (128, D)); dram_in('pe_T', (8, 128, T))
    dram_in('qm0', (1, B), f32); dram_in('qm1', (1, B), f32)
    dram_in('sab', (1, 9 * BL), f32)

    do = {}
    for m in MODS:
        do[f'gT_{m}'] = nc.dram_tensor(f'gT_{m}', [128, B], f32, kind="ExternalOutput")
        do[f'qT_{m}'] = nc.dram_tensor(f'qT_{m}', [P, 128, B], f32, kind="ExternalOutput")
        do[f'eT_{m}'] = nc.dram_tensor(f'eT_{m}', [128, B], f32, kind="ExternalOutput")
    do['alphaT'] = nc.dram_tensor('alphaT', [T, BL], f32, kind="ExternalOutput")
    import os
    if os.environ.get('KDBG'):
        for m in MODS:
            do[f'dbg_poolsT_{m}'] = nc.dram_tensor(f'dbg_poolsT_{m}', [128, 8 * 3 * BL],
                                                   bf16, kind="ExternalOutput")
            do[f'dbg_cT_{m}'] = nc.dram_tensor(f'dbg_cT_{m}', [128, 8 * BL], f32,
                                               kind="ExternalOutput")
            do[f'dbg_sp_{m}'] = nc.dram_tensor(f'dbg_sp_{m}', [128, 24 * BL], bf16,
                                               kind="ExternalOutput")
        do['dbg_sa'] = nc.dram_tensor('dbg_sa', [1, 9 * BL], f32, kind="ExternalOutput")

    ag = {}
    for m in MODS:
        ag[f'c_in_{m}'] = nc.dram_tensor(f'ag_c_in_{m}', [D, BL], f32)
        ag[f'c_out_{m}'] = nc.dram_tensor(f'ag_c_out_{m}', [N, D, BL], f32,
                                          addr_space="Shared")
        ag[f'q_in_{m}'] = nc.dram_tensor(f'ag_q_in_{m}', [128, B], bf16)
        ag[f'q_out_{m}'] = nc.dram_tensor(f'ag_q_out_{m}', [N, 128, B], bf16,
                                          addr_space="Shared")

    with tile.TileContext(nc) as tc:
        _emit(nc, tc, di, do, ag, mybir)

    nc.compile()
    _BUILT = nc
    return nc


def _emit(nc, tc, di, do, ag, mybir):
    import contextlib
    import os
    from concourse.masks import make_identity
    f32, bf16 = mybir.dt.float32, mybir.dt.bfloat16
    AF = mybir.ActivationFunctionType
    OP = mybir.AluOpType
    AX = mybir.AxisListType

    from bass_rust import add_dep_helper

    def chain(insts):
        for a_, b_ in zip(insts[1:], insts[:-1]):
            add_dep_helper(a_.ins, b_.ins, sync=False, reason="accum-order")

    KLIM = int(os.environ.get('KLIM', '99'))
    ctx = contextlib.ExitStack()
    with ctx:
        const = ctx.enter_context(tc.tile_pool(name="const", bufs=1))
        acts = ctx.enter_context(tc.tile_pool(name="acts", bufs=1))
        wp = ctx.enter_context(tc.tile_pool(name="wp", bufs=1))
        wp1 = ctx.enter_context(tc.tile_pool(name="wp1", bufs=1))
        gwp = ctx.enter_context(tc.tile_pool(name="gwp", bufs=3))
        ghp = ctx.enter_context(tc.tile_pool(name="ghp", bufs=4))
        work = ctx.enter_context(tc.tile_pool(name="work", bufs=2))
        mm = ctx.enter_context(tc.tile_pool(name="mm", bufs=2, space="PSUM"))
        tpp = ctx.enter_context(tc.tile_pool(name="tpp", bufs=2, space="PSUM"))
        sc = ctx.enter_context(tc.tile_pool(name="sc", bufs=2, space="PSUM"))
        plp = ctx.enter_context(tc.tile_pool(name="plp", bufs=1, space="PSUM"))

        ident = const.tile([128, 128], bf16)
        make_identity(nc, ident)

        def dma(dst_ap, src_ap):
            nc.sync.dma_start(dst_ap, src_ap)

        def load_small(name, shape, dt=f32):
            t = const.tile(list(shape), dt, tag=name)
            dma(t[:], di[name][:])
            return t

        def load_kt(name, pool, kt, width, dt=bf16, tag=None, idx=None, ap=None):
            """(kt,128,width) dram (or a sub-slice) -> sbuf (128, kt*width)."""
            t = pool.tile([128, kt * width], dt, tag=tag or name)
            if ap is None:
                ap = di[name][:] if idx is None else di[name][idx]
            dma(t[:].rearrange("p (k m) -> p k m", k=kt),
                ap.rearrange("k p m -> p k m"))
            return t

        bd = {m: load_small(f'bd_{m}', (128, 8)) for m in MODS}
        b1 = {m: load_small(f'b1_{m}', (128, 8)) for m in MODS}
        gbias = {}
        for m in MODS:
            for gru in 'gpe':
                gbias[f'{gru}{m}'] = load_small(f'b{gru}_{m}', (128, 4))
        hprevS = {m: load_small(f'hprevs_{m}', (128, B)) for m in MODS}
        q0S = {}
        for m in MODS:
            t = const.tile([128, P * B], f32, tag=f'q0s_{m}')
            dma(t[:].rearrange("p (a b) -> p a b", a=P),
                di[f'q0s_{m}'][:].rearrange("a p b -> p a b"))
            q0S[m] = t
        e0S = {m: load_small(f'e0s_{m}', (128, B)) for m in MODS}
        w3 = {m: load_kt(f'w3_{m}', const, 8, 3) for m in MODS}
        w3p = {m: load_kt(f'w3p_{m}', const, 8, 32) for m in MODS}
        wsa = {m: load_kt(f'wsa_{m}', const, 8, 1) for m in MODS}
        pe_t = load_small('pe_t', (128, D), dt=bf16)
        pe_T = load_kt('pe_T', const, 8, T)
        qm0 = load_small('qm0', (1, B))
        qm1 = load_small('qm1', (1, B))
        sab = load_small('sab', (1, 9 * BL))

        # party-select mask and broadcasts
        s_row = const.tile([1, B], mybir.dt.uint8)
        nc.vector.tensor_tensor(s_row[:], qm1[:], qm0[:], OP.is_gt)
        s_bc = const.tile([128, B], mybir.dt.uint8)
        nc.gpsimd.partition_broadcast(s_bc[:], s_row[:])
        qm_bc = const.tile([128, P * B], f32)
        nc.gpsimd.partition_broadcast(qm_bc[:, 0:B], qm0[:])
        nc.gpsimd.partition_broadcast(qm_bc[:, B:2 * B], qm1[:])

        # ---------- stage A: U'.T = (Wd.T).T @ U.T + bd  (full batch) ----------
        upT = {m: acts.tile([128, 8 * B], bf16, tag=f'upT_{m}') for m in MODS}
        for m in MODS:
            uT = load_kt(f'u_{m}', wp1, KTU[m], B, tag="uTs")
            wdT = load_kt(f'wd_{m}', wp1, KTU[m], D, tag="wds")
            for mt in range(8):
                ps = mm.tile([128, B], f32, tag="mm")
                chain([nc.tensor.matmul(
                    ps[:], wdT[:, kt * D + mt * 128: kt * D + (mt + 1) * 128],
                    uT[:, kt * B:(kt + 1) * B],
                    start=(kt == 0), stop=(kt == KTU[m] - 1))
                    for kt in range(KTU[m])])
                nc.scalar.activation(upT[m][:, mt * B:(mt + 1) * B], ps[:],
                                     AF.Identity, bias=bd[m][:, mt:mt + 1])

        # q0selT (bf16) for the g-GRU input
        q0selT = {m: acts.tile([128, 8 * B], bf16, tag=f'q0selT_{m}') for m in MODS}
        for m in MODS:
            q0p0 = load_kt(f'q0b_{m}', wp, 8, B, tag="q0str0", idx=0)
            q0p1 = load_kt(f'q0b_{m}', wp, 8, B, tag="q0str1", idx=1)
            for dt_ in range(8):
                sl = slice(dt_ * B, (dt_ + 1) * B)
                nc.vector.select(q0selT[m][:, sl], s_bc[:], q0p1[:, sl], q0p0[:, sl])

        if KLIM <= 1:
            return
        # ---------- attention (batch shard) ----------
        alphaT_acc = const.tile([T, BL], f32)
        poolsT = {m: acts.tile([128, 8 * 3 * BL], bf16, tag=f'poolsT_{m}', name=f'poolsT_{m}')
                  for m in MODS}
        NCH = BL // CH
        for m in MODS:
            # pe score fold: (3, T) = sum_d w3[d,k] pe_T[d,t]
            pe_ps = sc.tile([3, T], f32, tag="sc")
            chain([nc.tensor.matmul(pe_ps[:], w3[m][:, kt * 3:(kt + 1) * 3],
                                    pe_T[:, kt * T:(kt + 1) * T],
                                    start=(kt == 0), stop=(kt == 7))
                   for kt in range(8)])
            pe_sc = work.tile([3, T], f32, tag="pe_sc")
            nc.vector.tensor_copy(pe_sc[:], pe_ps[:])
            alTv = []
            PLT = plp.tile([128, 8 * 3 * BL], f32, tag="plp")
            for g in range(8):
                Sg = sc.tile([128, T], f32, tag="sc")
                chunk_tiles = {}
                ghTt = {}
                for j in range(4):
                    b = 4 * g + j
                    c, bi = b // CH, b % CH
                    if bi == 0:
                        tb = ghp.tile([128, CH * D], bf16, tag="ght",
                                      name=f"ght_{m}_{c}")
                        dma(tb[:].rearrange("p (b d) -> p b d", b=CH),
                            di[f'ght_{m}'][CH * c:CH * (c + 1)]
                            .rearrange("b p d -> p b d"))
                        chunk_tiles[c] = tb
                    tb = chunk_tiles[c]
                    # on-chip transpose (t,d) -> (d,t) for the score contraction
                    tp_ps = tpp.tile([128, 8 * T], bf16, tag="tpp", name=f"tp_{b}")
                    for dt_ in range(8):
                        nc.tensor.transpose(
                            tp_ps[:, dt_ * T:(dt_ + 1) * T],
                            tb[:, bi * D + dt_ * 128: bi * D + (dt_ + 1) * 128],
                            ident[:])
                    gt = work.tile([128, 8 * T], bf16, tag="ghTt", bufs=2)
                    nc.scalar.copy(gt[:, 0:4 * T], tp_ps[:, 0:4 * T])
                    nc.vector.tensor_copy(gt[:, 4 * T:8 * T], tp_ps[:, 4 * T:8 * T])
                    ghTt[j] = gt
                    chain([nc.tensor.matmul(
                        Sg[32 * j:32 * j + 32, :], w3p[m][:, kt * 32:(kt + 1) * 32],
                        gt[:, kt * T:(kt + 1) * T],
                        start=(kt == 0), stop=(kt == 7),
                        tile_position=(0, 32 * j), skip_group_check=True)
                        for kt in range(8)])
                    nc.vector.tensor_tensor(Sg[32 * j:32 * j + 3, :],
                                            Sg[32 * j:32 * j + 3, :], pe_sc[:], OP.add)
                # softmax over free dim (junk rows harmless)
                mx = work.tile([128, 1], f32, tag="mx")
                nc.vector.reduce_max(mx[:], Sg[:], axis=AX.X)
                nmx = work.tile([128, 1], f32, tag="nmx")
                nc.vector.tensor_scalar_mul(nmx[:], mx[:], -1.0)
                ex = work.tile([128, T], f32, tag="ex")
                sm = work.tile([128, 1], f32, tag="sm")
                nc.scalar.activation(ex[:], Sg[:], AF.Exp, bias=nmx[:], accum_out=sm[:])
                rec = work.tile([128, 1], f32, tag="rec")
                nc.vector.reciprocal(rec[:], sm[:])
                alb = work.tile([128, T], bf16, tag="alb")
                nc.vector.tensor_scalar_mul(alb[:], ex[:], rec[:])
                alT_ps = sc.tile([T, 128], bf16, tag="sc")
                nc.tensor.transpose(alT_ps[:], alb[:], ident[:])
                alTg = work.tile([T, 128], bf16, tag="alTg")
                nc.vector.tensor_copy(alTg[:], alT_ps[:])
                av = work.tile([T, 12], bf16, tag="alTv", bufs=8, name=f"av_{g}")
                nc.vector.tensor_copy(
                    av[:].rearrange("t (b k) -> t b k", k=3),
                    alTg[:].rearrange("t (b k) -> t b k", k=32)[:, :, 0:3])
                alTv.append(av)
                # pools for this group's 4 b (ght chunks still resident)
                for j in range(4):
                    b = 4 * g + j
                    c, bi = b // CH, b % CH
                    tb = chunk_tiles[c]
                    for dt_ in range(8):
                        nc.tensor.matmul(
                            PLT[:, dt_ * 3 * BL + 3 * b: dt_ * 3 * BL + 3 * b + 3],
                            tb[:, bi * D + dt_ * 128: bi * D + (dt_ + 1) * 128],
                            av[:, 3 * j:3 * j + 3],
                            start=True, stop=True, skip_group_check=True)
            # assemble packed alpha (T, 96) from the per-group tiles
            alT_packed = work.tile([T, 3 * BL], bf16, tag="alTp")
            for g in range(8):
                nc.vector.tensor_copy(alT_packed[:, 12 * g:12 * (g + 1)], alTv[g][:])
            # alpha output accumulation (sum the 3 k's per b)
            red = work.tile([T, BL], f32, tag="red")
            nc.vector.reduce_sum(red[:], alT_packed[:].rearrange("t (b k) -> t b k", k=3),
                                 axis=AX.X)
            if m == 't':
                nc.vector.tensor_copy(alphaT_acc[:], red[:])
            else:
                nc.vector.tensor_tensor(alphaT_acc[:], alphaT_acc[:], red[:], OP.add)
            # pe pool fold in separate psum, added at eviction
            pe_sb = work.tile([128, 8 * 3 * BL], bf16, tag="pe_sb")
            for dt_ in range(8):
                pe2 = sc.tile([128, 3 * BL], f32, tag="sc")
                nc.tensor.matmul(pe2[:], pe_t[:, dt_ * 128:(dt_ + 1) * 128],
                                 alT_packed[:], start=True, stop=True)
                nc.scalar.copy(pe_sb[:, dt_ * 3 * BL:(dt_ + 1) * 3 * BL], pe2[:])
            nc.vector.tensor_tensor(poolsT[m][:, 0:384], PLT[:, 0:384],
                                    pe_sb[:, 0:384], OP.add)
            nc.vector.tensor_tensor(poolsT[m][:, 384:768], PLT[:, 384:768],
                                    pe_sb[:, 384:768], OP.add)

        # ---------- self-att + dense1-3 (batch shard) ----------
        def pools_slice(src, kt, k):     # (128, BL) strided column view
            return poolsT[src][:, kt * 3 * BL:(kt + 1) * 3 * BL] \
                .rearrange("p (b k) -> p k b", k=3)[:, k]

        SA = sc.tile([1, 9 * BL], f32, tag="sc")
        for ri, m in enumerate(MODS):
            for si, (src, k) in enumerate(RECV_SLOTS[m]):
                out_sl = SA[0:1, ri * 3 * BL + si * BL: ri * 3 * BL + (si + 1) * BL]
                chain([nc.tensor.matmul(out_sl, wsa[m][:, kt:kt + 1],
                                        pools_slice(src, kt, k),
                                        start=(kt == 0), stop=(kt == 7),
                                        skip_group_check=True)
                       for kt in range(8)])
        sa_e = work.tile([1, 9 * BL], f32, tag="sa_e")
        nc.vector.tensor_tensor(sa_e[:], SA[:], sab[:], OP.add)
        nc.scalar.activation(sa_e[:], sa_e[:], AF.Exp)
        scale_row = work.tile([1, 9 * BL], f32, tag="scale_row")
        for ri, m in enumerate(MODS):
            base = ri * 3 * BL
            ssum = work.tile([1, BL], f32, tag="ssum")
            nc.vector.tensor_tensor(ssum[:], sa_e[0:1, base:base + BL],
                                    sa_e[0:1, base + BL:base + 2 * BL], OP.add)
            nc.vector.tensor_tensor(ssum[:], ssum[:],
                                    sa_e[0:1, base + 2 * BL:base + 3 * BL], OP.add)
            srec = work.tile([1, BL], f32, tag="srec")
            nc.vector.reciprocal(srec[:], ssum[:])
            for si in range(3):
                sl = slice(base + si * BL, base + (si + 1) * BL)
                nc.vector.tensor_tensor(scale_row[0:1, sl], sa_e[0:1, sl], srec[:],
                                        OP.mult)
        if 'dbg_sa' in do:
            dma(do['dbg_sa'][:], scale_row[:])
        sc_bc = const.tile([128, 9 * BL], f32)
        nc.gpsimd.partition_broadcast(sc_bc[:], scale_row[:])
        sp = {}
        for ri, m in enumerate(MODS):
            sp[m] = acts.tile([128, 24 * BL], bf16, tag=f'sp_{m}')
            for si, (src, k) in enumerate(RECV_SLOTS[m]):
                for kt in range(8):
                    j = si * 8 + kt
                    nc.vector.tensor_tensor(
                        sp[m][:, j * BL:(j + 1) * BL], pools_slice(src, kt, k),
                        sc_bc[:, ri * 3 * BL + si * BL: ri * 3 * BL + (si + 1) * BL],
                        OP.mult)
        # dense1-3 + AG1
        for m in MODS:
            cT_loc = acts.tile([128, 8 * BL], f32, tag=f'cT_{m}')
            ps = mm.tile([128, 8 * BL], f32, tag="mm")
            prev_mt = [None] * 8
            for kt in range(24):
                w1t = gwp.tile([128, D], bf16, tag="w1t")
                dma(w1t[:], di[f'w1_{m}'][kt])
                for mt in range(8):
                    mmins = nc.tensor.matmul(ps[:, mt * BL:(mt + 1) * BL],
                                             w1t[:, mt * 128:(mt + 1) * 128],
                                             sp[m][:, kt * BL:(kt + 1) * BL],
                                             start=(kt == 0), stop=(kt == 23),
                                             skip_group_check=True)
                    if prev_mt[mt] is not None:
                        add_dep_helper(mmins.ins, prev_mt[mt].ins, sync=False,
                                       reason="kt-order")
                    prev_mt[mt] = mmins
            for mt in range(8):
                nc.scalar.activation(cT_loc[:, mt * BL:(mt + 1) * BL],
                                     ps[:, mt * BL:(mt + 1) * BL],
                                     AF.Identity, bias=b1[m][:, mt:mt + 1])
            if f'dbg_cT_{m}' in do:
                dma(do[f'dbg_cT_{m}'][:], cT_loc[:])
                dma(do[f'dbg_sp_{m}'][:], sp[m][:])
            dma(ag[f'c_in_{m}'][:].rearrange("(dt p) b -> p dt b", p=128),
                cT_loc[:].rearrange("p (dt b) -> p dt b", dt=8))
            if os.environ.get('KNOCC'):
                for _r in range(N):
                    nc.sync.dma_start(ag[f'c_out_{m}'][_r], ag[f'c_in_{m}'][:])
            else:
                nc.gpsimd.collective_compute(
                    "AllGather", OP.bypass, replica_groups=[list(range(N))],
                    ins=[ag[f'c_in_{m}'][:]], outs=[ag[f'c_out_{m}'][:]])

        if KLIM <= 3:
            return
        # ---------- p-GRU part 1: gi from U' (runs during AG1) ----------
        pgi_sb = {}
        for m in MODS:
            pgi_sb[m] = acts.tile([128, 3 * B], f32, tag=f'pgi_{m}')
            for g in range(3):
                wgi = load_kt(f'wpi_{m}', gwp, 8, 128, tag="wg8",
                              ap=di[f'wpi_{m}'][:][g, 0:8])
                ps = mm.tile([128, B], f32, tag="mm")
                chain([nc.tensor.matmul(ps[:], wgi[:, kt * 128:(kt + 1) * 128],
                                        upT[m][:, kt * B:(kt + 1) * B],
                                        start=(kt == 0), stop=(kt == 7))
                       for kt in range(8)])
                nc.scalar.copy(pgi_sb[m][:, g * B:(g + 1) * B], ps[:])

        if KLIM <= 4:
            return
        # ---------- g-GRUs (more AG1 filler) ----------
        for m in MODS:
            hprevT = load_kt(f'hprev_{m}', wp, 8, B, tag="hpT")
            psums = []
            for g in range(3):
                wgi = load_kt(f'wgi_{m}', gwp, 16, 128, tag="wg16", idx=g)
                wgh = load_kt(f'wgh_{m}', gwp, 8, 128, tag="wg8", idx=g)
                if g < 2:
                    ps = mm.tile([128, B], f32, tag="mm")
                    grp = [nc.tensor.matmul(
                        ps[:], wgi[:, kt * 128:(kt + 1) * 128],
                        (upT[m] if kt < 8 else q0selT[m])[:, (kt % 8) * B:(kt % 8 + 1) * B],
                        start=(kt == 0), stop=False, skip_group_check=True)
                        for kt in range(16)]
                    grp += [nc.tensor.matmul(
                        ps[:], wgh[:, kt * 128:(kt + 1) * 128],
                        hprevT[:, kt * B:(kt + 1) * B],
                        start=False, stop=(kt == 7), skip_group_check=True)
                        for kt in range(8)]
                    chain(grp)
                    psums.append(ps)
                else:
                    ps_ni = mm.tile([128, B], f32, tag="mm")
                    chain([nc.tensor.matmul(
                        ps_ni[:], wgi[:, kt * 128:(kt + 1) * 128],
                        (upT[m] if kt < 8 else q0selT[m])[:, (kt % 8) * B:(kt % 8 + 1) * B],
                        start=(kt == 0), stop=(kt == 15))
                        for kt in range(16)])
                    ps_nh = mm.tile([128, B], f32, tag="mm")
                    chain([nc.tensor.matmul(
                        ps_nh[:], wgh[:, kt * 128:(kt + 1) * 128],
                        hprevT[:, kt * B:(kt + 1) * B],
                        start=(kt == 0), stop=(kt == 7))
                        for kt in range(8)])
                    psums += [ps_ni, ps_nh]
            _gru_tail(nc, work, AF, OP, f32, psums[0], psums[1], psums[2], psums[3],
                      gbias[f'g{m}'], hprevS[m][:], do[f'gT_{m}'][:], dma)

        if KLIM <= 5:
            return
        # ---------- p-GRU part 2: needs AG1 ----------
        for m in MODS:
            cTb = wp.tile([128, 8 * B], bf16, tag="cTb")
            for dt_ in range(8):
                cf = work.tile([128, B], f32, tag="cf")
                dma(cf[:].rearrange("p (r b) -> p r b", r=N),
                    ag[f'c_out_{m}'][:, dt_ * 128:(dt_ + 1) * 128, :]
                    .rearrange("r p b -> p r b"))
                nc.vector.tensor_copy(cTb[:, dt_ * B:(dt_ + 1) * B], cf[:])
            qparts = []
            for par in range(P):
                q0Tp = load_kt(f'q0b_{m}', wp, 8, B, tag="q0pT", idx=par)
                gps = []
                for g in range(3):
                    wci = load_kt(f'wpi_{m}', gwp, 8, 128, tag="wg8",
                                  ap=di[f'wpi_{m}'][:][g, 8:16])
                    wph = load_kt(f'wph_{m}', gwp, 8, 128, tag="wg8", idx=g)
                    if g < 2:
                        # r/z: c-part + h-part accumulate together
                        ps = mm.tile([128, B], f32, tag="mm")
                        grp = [nc.tensor.matmul(
                            ps[:], wci[:, kt * 128:(kt + 1) * 128],
                            cTb[:, kt * B:(kt + 1) * B],
                            start=(kt == 0), stop=False, skip_group_check=True)
                            for kt in range(8)]
                        grp += [nc.tensor.matmul(
                            ps[:], wph[:, kt * 128:(kt + 1) * 128],
                            q0Tp[:, kt * B:(kt + 1) * B],
                            start=False, stop=(kt == 7), skip_group_check=True)
                            for kt in range(8)]
                        chain(grp)
                        gps.append(ps)
                    else:
                        ps_nc = mm.tile([128, B], f32, tag="mm")
                        chain([nc.tensor.matmul(
                            ps_nc[:], wci[:, kt * 128:(kt + 1) * 128],
                            cTb[:, kt * B:(kt + 1) * B],
                            start=(kt == 0), stop=(kt == 7))
                            for kt in range(8)])
                        ps_nh = mm.tile([128, B], f32, tag="mm")
                        chain([nc.tensor.matmul(
                            ps_nh[:], wph[:, kt * 128:(kt + 1) * 128],
                            q0Tp[:, kt * B:(kt + 1) * B],
                            start=(kt == 0), stop=(kt == 7))
                            for kt in range(8)])
                        gps += [ps_nc, ps_nh]
                bias4 = gbias[f'p{m}']
                gisb = pgi_sb[m]
                r_ = work.tile([128, B], f32, tag="r_")
                nc.vector.tensor_tensor(r_[:], gisb[:, 0:B], gps[0][:], OP.add)
                nc.scalar.activation(r_[:], r_[:], AF.Sigmoid, bias=bias4[:, 0:1])
                z_ = work.tile([128, B], f32, tag="z_")
                nc.vector.tensor_tensor(z_[:], gisb[:, B:2 * B], gps[1][:], OP.add)
                nc.scalar.activation(z_[:], z_[:], AF.Sigmoid, bias=bias4[:, 1:2])
                hb = work.tile([128, B], f32, tag="hb")
                nc.scalar.activation(hb[:], gps[3][:], AF.Identity, bias=bias4[:, 3:4])
                nc.vector.tensor_tensor(hb[:], r_[:], hb[:], OP.mult)
                t2 = work.tile([128, B], f32, tag="t2")
                nc.vector.tensor_tensor(t2[:], gisb[:, 2 * B:3 * B], gps[2][:], OP.add)
                nc.vector.tensor_tensor(t2[:], t2[:], hb[:], OP.add)
                n_ = work.tile([128, B], f32, tag="n_")
                nc.scalar.activation(n_[:], t2[:], AF.Tanh, bias=bias4[:, 2:3])
                q0sh = q0S[m][:, par * B:(par + 1) * B]
                d_ = work.tile([128, B], f32, tag="d_")
                nc.vector.tensor_tensor(d_[:], q0sh, n_[:], OP.subtract)
                nc.vector.tensor_tensor(d_[:], z_[:], d_[:], OP.mult)
                hp = work.tile([128, B], f32, tag=f"hp{par}")
                nc.vector.tensor_tensor(hp[:], n_[:], d_[:], OP.add)   # qs
                dq = work.tile([128, B], f32, tag="dq")
                nc.vector.tensor_tensor(dq[:], hp[:], q0sh, OP.subtract)
                nc.vector.tensor_tensor(dq[:], qm_bc[:, par * B:(par + 1) * B],
                                        dq[:], OP.mult)
                qf = work.tile([128, B], f32, tag=f"qf{par}")
                nc.vector.tensor_tensor(qf[:], q0sh, dq[:], OP.add)
                dma(do[f'qT_{m}'][par], qf[:])
                qparts.append(qf)
            qself = work.tile([128, B], f32, tag="qself")
            nc.vector.select(qself[:], s_bc[:], qparts[1][:], qparts[0][:])
            qsel = work.tile([128, B], bf16, tag="qsel")
            nc.vector.tensor_copy(qsel[:], qself[:])
            dma(ag[f'q_in_{m}'][:], qsel[:])
            if os.environ.get('KNOCC'):
                for _r in range(N):
                    nc.sync.dma_start(ag[f'q_out_{m}'][_r], ag[f'q_in_{m}'][:])
            else:
                nc.gpsimd.collective_compute(
                    "AllGather", OP.bypass, replica_groups=[list(range(N))],
                    ins=[ag[f'q_in_{m}'][:]], outs=[ag[f'q_out_{m}'][:]])

        if KLIM <= 6:
            return
        # ---------- e-GRU: gh part first (AG2 filler), then gi ----------
        egh_sb = {}
        for m in MODS:
            e0T = load_kt(f'e0b_{m}', wp, 8, B, tag="e0T")
            egh_sb[m] = acts.tile([128, 3 * B], f32, tag=f'egh_{m}')
            for g in range(3):
                weh = load_kt(f'weh_{m}', gwp, 8, 128, tag="wg8", idx=g)
                ps = mm.tile([128, B], f32, tag="mm")
                chain([nc.tensor.matmul(ps[:], weh[:, kt * 128:(kt + 1) * 128],
                                        e0T[:, kt * B:(kt + 1) * B],
                                        start=(kt == 0), stop=(kt == 7))
                       for kt in range(8)])
                nc.scalar.copy(egh_sb[m][:, g * B:(g + 1) * B], ps[:])
        for m in MODS:
            qTf = wp.tile([128, 8 * B], bf16, tag="qTf")
            dma(qTf[:].rearrange("p (r b) -> p r b", r=N),
                ag[f'q_out_{m}'][:].rearrange("r p b -> p r b"))
            gps = []
            for g in range(3):
                wei = load_kt(f'wei_{m}', gwp, 8, 128, tag="wg8", idx=g)
                ps = mm.tile([128, B], f32, tag="mm")
                chain([nc.tensor.matmul(ps[:], wei[:, kt * 128:(kt + 1) * 128],
                                        qTf[:, kt * B:(kt + 1) * B],
                                        start=(kt == 0), stop=(kt == 7))
                       for kt in range(8)])
                gps.append(ps)
            bias4 = gbias[f'e{m}']
            egh = egh_sb[m]
            r_ = work.tile([128, B], f32, tag="r_")
            nc.vector.tensor_tensor(r_[:], egh[:, 0:B], gps[0][:], OP.add)
            nc.scalar.activation(r_[:], r_[:], AF.Sigmoid, bias=bias4[:, 0:1])
            z_ = work.tile([128, B], f32, tag="z_")
            nc.vector.tensor_tensor(z_[:], egh[:, B:2 * B], gps[1][:], OP.add)
            nc.scalar.activation(z_[:], z_[:], AF.Sigmoid, bias=bias4[:, 1:2])
            hb = work.tile([128, B], f32, tag="hb")
            nc.scalar.activation(hb[:], egh[:, 2 * B:3 * B], AF.Identity,
                                 bias=bias4[:, 3:4])
            nc.vector.tensor_tensor(hb[:], r_[:], hb[:], OP.mult)
            t2 = work.tile([128, B], f32, tag="t2")
            nc.vector.tensor_tensor(t2[:], gps[2][:], hb[:], OP.add)
            n_ = work.tile([128, B], f32, tag="n_")
            nc.scalar.activation(n_[:], t2[:], AF.Tanh, bias=bias4[:, 2:3])
            d_ = work.tile([128, B], f32, tag="d_")
            nc.vector.tensor_tensor(d_[:], e0S[m][:], n_[:], OP.subtract)
            nc.vector.tensor_tensor(d_[:], z_[:], d_[:], OP.mult)
            hp = work.tile([128, B], f32, tag="hp0")
            nc.vector.tensor_tensor(hp[:], n_[:], d_[:], OP.add)
            dma(do[f'eT_{m}'][:], hp[:])

        dma(do['alphaT'][:], alphaT_acc[:])


def run(in_maps, trace=False):
    from concourse import bass_utils
    nc = build_module()
    return bass_utils.run_bass_kernel_spmd(
        nc, in_maps, core_ids=list(range(N)), trace=trace)


def assemble(results):
    outs = []
    for m in MODS:
        g = np.concatenate([results[r][f'gT_{m}'] for r in range(N)], axis=0).T
        q = np.concatenate([results[r][f'qT_{m}'] for r in range(N)], axis=1) \
            .transpose(2, 0, 1)
        e = np.concatenate([results[r][f'eT_{m}'] for r in range(N)], axis=0).T
        outs.append((np.ascontiguousarray(g), np.ascontiguousarray(q),
                     np.ascontiguousarray(e)))
    alpha = np.concatenate([results[r]['alphaT'] for r in range(N)], axis=1) \
        .T[:, None, :]
    (g_t, q_t, e_t), (g_v, q_v, e_v), (g_a, q_a, e_a) = outs
    return (g_t, q_t, e_t, g_v, q_v, e_v, g_a, q_a, e_a,
            np.ascontiguousarray(alpha))


def kernel(**inputs):
    in_maps = prep_inputs(**inputs)
    res = run(in_maps)
    return assemble(res.results)


def _gru_tail(nc, work, AF, OP, f32, ps_r, ps_z, ps_ni, ps_nh, bias4, h_f32,
              out_dram_ap, dma):
    """Gate math for a GRU whose r/z psums already hold gi+gh, n split in two."""
    r_ = work.tile([128, B], f32, tag="r_")
    nc.scalar.activation(r_[:], ps_r[:], AF.Sigmoid, bias=bias4[:, 0:1])
    z_ = work.tile([128, B], f32, tag="z_")
    nc.scalar.activation(z_[:], ps_z[:], AF.Sigmoid, bias=bias4[:, 1:2])
    hb = work.tile([128, B], f32, tag="hb")
    nc.scalar.activation(hb[:], ps_nh[:], AF.Identity, bias=bias4[:, 3:4])
    nc.vector.tensor_tensor(hb[:], r_[:], hb[:], OP.mult)
    t2 = work.tile([128, B], f32, tag="t2")
    nc.vector.tensor_tensor(t2[:], ps_ni[:], hb[:], OP.add)
    n_ = work.tile([128, B], f32, tag="n_")
    nc.scalar.activation(n_[:], t2[:], AF.Tanh, bias=bias4[:, 2:3])
    d_ = work.tile([128, B], f32, tag="d_")
    nc.vector.tensor_tensor(d_[:], h_f32, n_[:], OP.subtract)
    nc.vector.tensor_tensor(d_[:], z_[:], d_[:], OP.mult)
    hp = work.tile([128, B], f32, tag="hp")
    nc.vector.tensor_tensor(hp[:], n_[:], d_[:], OP.add)
    dma(out_dram_ap, hp[:])
    return hp
